# revision 13
# baseline (speedup 1.0000x reference)
"""Trainium2 Bass kernel for nn_ObjectDetector_30451318129100 (CenterNet-style
peak decode: 3x3 max-pool NMS over a [1,512,512,80] heatmap -> global top-128
peaks -> per-peak (x,y) refinement + (w,h) gather).

Strategy (8 NeuronCores, single image):
  * Shard the heatmap row-wise: core k gets rows [64k, 64k+64) -> a contiguous
    10.5 MB flat slice (layout is [H, W, C] row-major, C fastest).
  * Device kernel (identical SPMD program): stream the shard through SBUF and
    emit the max of every 512-element flat segment ([128, 40] f32 per core).
    This is the only full-data pass -> HBM-bandwidth bound, which is the
    roofline for this problem (only hmap needs a full read; rreg/bbox are
    gathered at just 128 locations).
  * Host merge ("merge partial top-Ks"): segment maxima are exact upper
    bounds, so we scan segments in descending-max order, extract and verify
    candidate peaks (exact 3x3 NMS test), and stop as soon as no unprocessed
    segment can beat the current 128th peak. Provably exact for any input.
    Then top-128 by (value desc, flat index asc) — jax.lax.top_k tie order —
    and the tiny 128-point gathers from rreg/bbox.
"""

from contextlib import ExitStack

import numpy as np

import concourse.bass as bass
import concourse.mybir as mybir
from concourse.bass_utils import run_bass_kernel_spmd

# ---- problem constants (hardcoded; kernel.py must be self-contained) ----
H = 512
W = 512
C = 80
K = 128
STRIDE = 4
PEAK_NMS = 0.1

N_CORES = 8
ROWS_PER_CORE = H // N_CORES            # 64
ELEMS_PER_CORE = ROWS_PER_CORE * W * C  # 2,621,440
P = 128                                 # SBUF partitions
FREE = ELEMS_PER_CORE // P              # 20480 elements per partition
SEG = 512                               # flat segment size for the device max
N_SEGS = FREE // SEG                    # 40 per partition
# ~1-2 MB loads keep the DMA stream at line rate; the taper at the end keeps
# the post-last-byte tail short (each chunk's reduce is released ~1.3 us after
# its DMA lands due to completion-sem latency, so only the END shape matters)
CHUNKS = [4096] + [2048] * 6 + [1536, 1024, 512, 512, 512]
assert sum(CHUNKS) == FREE
N_CHUNKS = len(CHUNKS)

_nc_cache = None


def _build_nc(reps=1):
    """Per-core SPMD program: segmented max-reduce of the heatmap shard.

    Raw Bass (no Tile): chunk loads issued on the SP HWDGE ring, the vector
    engine reduces each chunk as its DMA lands, one final store. Explicit
    sems keep every instruction at <=1 sync wait (HW cap on this target is
    tiny; Tile's tail drain exceeds it).

    reps>1 repeats the whole streaming body (reusing the same SBUF tiles)
    so steady-state per-iteration HW time can be measured by differencing —
    functionally identical output (last rep wins, all reps identical).
    """
    nc = bass.Bass()
    hs = nc.declare_dram_parameter("hs", [P, FREE], mybir.dt.float32, isOutput=False)
    segmax = nc.declare_dram_parameter(
        "segmax", [P, N_SEGS], mybir.dt.float32, isOutput=True
    )
    offs = [0]
    for c in CHUNKS:
        offs.append(offs[-1] + c)
    with ExitStack() as ctx:
        tiles = [
            ctx.enter_context(
                nc.sbuf_tensor(f"chunk{i}", [P, CHUNKS[i]], mybir.dt.float32)
            )
            for i in range(N_CHUNKS)
        ]
        out_t = ctx.enter_context(
            nc.sbuf_tensor("out_t", [P, N_SEGS], mybir.dt.float32)
        )
        # one completion sem per chunk: concurrent DMAs finish out of order
        # across the 16 SDMA engines, so a shared counter would be racy
        load_sems = [
            ctx.enter_context(nc.semaphore(f"load_sem{i}")) for i in range(N_CHUNKS)
        ]
        red_sem = ctx.enter_context(nc.semaphore("red_sem"))
        st_sem = ctx.enter_context(nc.semaphore("st_sem"))
        block = ctx.enter_context(nc.Block())

        @block.sync
        def _(sync):
            for r in range(reps):
                for i in range(N_CHUNKS):
                    if r > 0:
                        # reuse tile i only after its previous reduce consumed it
                        sync.wait_ge(red_sem, (r - 1) * N_CHUNKS + i + 1)
                    sync.dma_start(
                        out=tiles[i][:], in_=hs[:, offs[i] : offs[i + 1]]
                    ).then_inc(load_sems[i], 16)
            sync.wait_ge(red_sem, reps * N_CHUNKS)
            sync.dma_start(out=segmax[:], in_=out_t[:]).then_inc(st_sem, 16)
            sync.wait_ge(st_sem, 16)

        @block.vector
        def _(vector):
            for r in range(reps):
                for i in range(N_CHUNKS):
                    vector.wait_ge(load_sems[i], 16 * (r + 1))
                    vector.reduce_max(
                        out=out_t[:, offs[i] // SEG : offs[i + 1] // SEG],
                        in_=tiles[i].rearrange("p (s k) -> p s k", k=SEG),
                        axis=mybir.AxisListType.X,
                    ).then_inc(red_sem, 1)

    return nc


def _device_segmax(hmap, trace=False):
    """Run the SPMD kernel; returns ([total_segments] f32 maxima, results obj).

    Segment g (0 <= g < 8*128*40) covers flat hmap elements
    [g*512, (g+1)*512): core k = g // (128*40), partition p = (g // 40) % 128,
    seg s = g % 40 map to flat offset k*ELEMS_PER_CORE + p*FREE + s*SEG —
    which is exactly g*SEG because the shard layout is contiguous.
    """
    global _nc_cache
    if _nc_cache is None:
        _nc_cache = _build_nc()
    nc = _nc_cache
    shards = np.ascontiguousarray(
        hmap.reshape(N_CORES, P, FREE)
    )  # [8, 128, 20480], row-major contiguous split
    in_maps = [{"hs": shards[k]} for k in range(N_CORES)]
    res = run_bass_kernel_spmd(nc, in_maps, list(range(N_CORES)), trace=trace)
    segmax = np.concatenate(
        [res.results[k]["segmax"].reshape(-1) for k in range(N_CORES)]
    )
    return segmax, res


def _host_complete(hmap3, rreg, bbox, segmax):
    """Exact top-K completion from per-segment maxima (upper bounds)."""
    flat = hmap3.reshape(-1)
    nseg = segmax.shape[0]
    order = np.argsort(-segmax, kind="stable")

    T = 256
    while True:
        T = min(T, nseg)
        segs = order[:T]
        v_rest = segmax[order[T]] if T < nseg else -np.inf
        floor = max(np.float32(PEAK_NMS), v_rest)
        base = segs[:, None] * SEG + np.arange(SEG)[None, :]
        vals = flat[base]
        m = vals > floor
        cand_flat = base[m]
        cand_val = vals[m]
        c = cand_flat % C
        x = (cand_flat // C) % W
        y = cand_flat // (C * W)
        is_peak = np.ones(cand_flat.shape[0], dtype=bool)
        for dy in (-1, 0, 1):
            for dx in (-1, 0, 1):
                if dy == 0 and dx == 0:
                    continue
                yy = y + dy
                xx = x + dx
                valid = (yy >= 0) & (yy < H) & (xx >= 0) & (xx < W)
                nb = np.where(
                    valid,
                    hmap3[np.clip(yy, 0, H - 1), np.clip(xx, 0, W - 1), c],
                    -np.inf,
                )
                is_peak &= cand_val >= nb
        pv = cand_val[is_peak]
        pf = cand_flat[is_peak]
        if pv.shape[0] >= K or T == nseg:
            break
        T *= 2

    if pv.shape[0] < K:
        # fewer than K peaks above the highest provable floor anywhere ->
        # fall back to the exact dense path (never hit for real inputs)
        return _dense_fallback(hmap3, rreg, bbox)

    sel = np.lexsort((pf, -pv))[:K]
    return _finalize(hmap3, rreg, bbox, pv[sel], pf[sel])


def _dense_fallback(hmap3, rreg, bbox):
    """Full numpy reference path (exact, slow); safety net only."""
    pad = np.pad(hmap3, ((1, 1), (1, 1), (0, 0)), constant_values=-np.inf)
    pooled = hmap3.copy()
    for dy in (0, 1, 2):
        for dx in (0, 1, 2):
            np.maximum(pooled, pad[dy : dy + H, dx : dx + W, :], out=pooled)
    keep = (hmap3 == pooled) & (hmap3 > PEAK_NMS)
    masked = np.where(keep, hmap3, np.float32(-1.0)).reshape(-1)
    idx = np.lexsort((np.arange(masked.shape[0]), -masked))[:K]
    return _finalize(hmap3, rreg, bbox, masked[idx], idx)


def _finalize(hmap3, rreg, bbox, scores, fidx):
    cls = (fidx % C).astype(np.int32)
    x = ((fidx // C) % W).astype(np.int32)
    y = (fidx // (C * W)).astype(np.int32)
    r0 = rreg[0]
    refinements = np.stack([r0[y, x, 2 * cls + 1], r0[y, x, 2 * cls]], axis=-1)
    box_params = bbox[0][y, x, :] * np.float32(STRIDE)
    refined = (
        np.stack([y, x], axis=-1).astype(np.float32) + refinements
    ) * np.float32(STRIDE)
    centroids = np.round(refined[:, ::-1]).astype(np.int32)
    return centroids, box_params, cls, scores.astype(np.float32)


def kernel(hmap, rreg, bbox):
    hmap = np.asarray(hmap, dtype=np.float32)
    rreg = np.asarray(rreg, dtype=np.float32)
    bbox = np.asarray(bbox, dtype=np.float32)
    segmax, _ = _device_segmax(hmap)
    return _host_complete(hmap[0], rreg, bbox, segmax)
